# revision 42
# baseline (speedup 1.0000x reference)
"""Trainium2 Bass kernel for nn_DiscriminativeLoss (segment_reduce).

Strategy (pure data parallel, 8 cores = 4 images x 2 half-images):
  Each core handles 256 output rows (half of a 512x512 image) for one image.
  On device (per core):
    - stage 1 (PE): row-upsample  B[w, c, R] = sum_h X[h,c,w] * Ur[h,R]
    - stage 2 (PE): col-upsample  EU[CC, c, R] = sum_w Uc[w,CC] * B[w,c,R]
    - evac (ACT+DVE split, batched 2 PSUM banks/op): EU copy -> ALL[:, r, 0:32]
      and EU^2 (DVE mult / ACT square) -> ALL[:, r, 32:64], r-major bf16.
    - PE segment-reduce: for each 128-pixel tile (row r, col chunk m):
        acc[k, :] += onehot[pix, k]^T @ ALL[pix, r, :]   (N=64)
      one-hot ships from HOST as exact fp8 0/1 (pure function of the label
      input); 4 independent accumulators in PE col-groups (tile_position).
  Host: per-class counts via bincount (exact), then combines the 8 (19,64)
  partials into per-class count / sum / sum-of-squares and evaluates the
  tiny closed-form loss exactly as the reference.

The bilinear-resize weight matrix replicates jax.image.resize (triangle
kernel, half-pixel centers, edge renormalization) and is fed to the device,
so the upsample is the exact same linear operator as the reference.
"""

import numpy as np

N_IMAGES = 4
C = 32
HIN = WIN = 128
HOUT = WOUT = 512
K = 19          # n_classes
RHALF = 256     # output rows per core
HS = 65         # input rows per core (with halo)
NV = 2 * C      # rhs value columns: 32 emb + 32 emb^2
MCH = 4         # output column chunks of 128
NCORES = 8
AUXW = RHALF + WOUT  # 768: WR cols then WC cols


def _resize_weight_mat(in_size, out_size):
    """(out, in) weight matrix of jax.image.resize(..., method='bilinear')."""
    scale = out_size / in_size
    inv_scale = 1.0 / scale
    sample_f = (np.arange(out_size, dtype=np.float32) + 0.5) * inv_scale - 0.5
    x = np.abs(sample_f[None, :] - np.arange(in_size, dtype=np.float32)[:, None])
    weights = np.maximum(0, 1 - x)
    total = weights.sum(axis=0, keepdims=True)
    weights = np.where(
        np.abs(total) > 1000.0 * np.finfo(np.float32).eps,
        weights / np.where(total != 0, total, 1),
        0,
    )
    keep = (sample_f >= -0.5) & (sample_f <= in_size - 0.5)
    weights = np.where(keep[None, :], weights, 0)
    return np.ascontiguousarray(weights.T.astype(np.float32))  # (out, in)


def _trace_device_kernel(nc, tile, mybir, x, aux, oh, out):
    from contextlib import ExitStack

    f32 = mybir.dt.float32
    bf16 = mybir.dt.bfloat16
    f8 = mybir.dt.float8e4
    with tile.TileContext(nc) as tc:
        with ExitStack() as ctx:
            consts = ctx.enter_context(tc.tile_pool(name="consts", bufs=1))
            AUX = consts.tile([WIN, AUXW], bf16)
            nc.sync.dma_start(out=AUX[:, 0:RHALF], in_=aux[:, 0:RHALF])
            X_sb = consts.tile([HS, C, WIN], bf16)
            nc.sync.dma_start(out=X_sb[:], in_=x[:])
            nc.scalar.dma_start(out=AUX[:, RHALF:AUXW], in_=aux[:, RHALF:AUXW])
            WR_sb = AUX[0:HS, 0:RHALF]
            # one-hot ships as uint8 bytes (jax bridge lacks fp8e4m3);
            # bitcast to fp8 at the matmul weight read.
            OH_sb = consts.tile([WIN, MCH, RHALF * K], mybir.dt.uint8)
            for m in range(MCH):
                nc.sync.dma_start(out=OH_sb[:, m, :], in_=oh[m, :, :])

            B = consts.tile([WIN, C, RHALF], bf16)
            # 2-bank psum tiles: two matmuls (one per bank half) per tile,
            # then one batched 2-bank evacuation per engine.
            ps2 = ctx.enter_context(
                tc.tile_pool(name="ps2", bufs=2, space="PSUM")
            )
            # warm the PE HAM clock gate during the input-DMA wait:
            # dependency-free matmuls on a memset tile bring the PE to
            # 2.4 GHz before stage 1 issues.
            WARM = consts.tile([WIN, WIN], bf16)
            nc.vector.memset(WARM[:], 0.0)
            for w in range(20):
                pw = ps2.tile([WIN, 4, RHALF], f32, tag="ps2")
                nc.tensor.matmul(
                    pw[0:1, 0, 0:WIN], WARM[:, 0:1], WARM[:],
                    start=True, stop=True,
                )

            def emit_stage1_tile(t):
                """Row-upsample channels 4t..4t+4 into B."""
                c = 4 * t
                p = ps2.tile([WIN, 4, RHALF], f32, tag="ps2")
                for j in range(4):
                    nc.tensor.matmul(
                        p[:, j, :], X_sb[:, c + j, :], WR_sb[:],
                        start=True, stop=True,
                    )
                if t % 2 == 0:
                    nc.vector.tensor_copy(B[:, c : c + 4, :], p[:])
                else:
                    nc.scalar.copy(B[:, c : c + 4, :], p[:])

            def emit_stage2_tile(ALLm, m, t):
                """Col-upsample channels 4t..4t+4 of chunk m into ALLm,
                evacuate (copy) + square. DVE copies 2/8, ACT 6/8; all
                squares on DVE (2x on contiguous bf16)."""
                cg = 4 * t
                WC_m = AUX[:, RHALF + m * WIN : RHALF + (m + 1) * WIN]
                p = ps2.tile([WIN, 4, RHALF], f32, tag="ps2")
                for j in range(0, 4, 2):
                    nc.tensor.matmul(
                        p[:, j : j + 2, :],
                        WC_m,
                        B[:, cg + j : cg + j + 2, :],
                        start=True, stop=True,
                    )
                euv = ALLm[:, cg : cg + 4, :]
                sqv = ALLm[:, C + cg : C + cg + 4, :]
                if t == 0:
                    nc.vector.tensor_copy(euv, p[:])
                else:
                    nc.scalar.copy(euv, p[:])
                nc.vector.tensor_tensor(
                    out=sqv, in0=euv, in1=euv, op=mybir.AluOpType.mult
                )

            allpool = ctx.enter_context(tc.tile_pool(name="allpool", bufs=2))
            accpool = ctx.enter_context(
                tc.tile_pool(name="accpool", bufs=1, space="PSUM")
            )
            outpool = ctx.enter_context(tc.tile_pool(name="outpool", bufs=1))
            # 4 independent accumulators in col-groups 0..3 of the PE array
            # (tile_position packing): group g = r % 4 accumulates into
            # partitions [32g, 32g+19) of its own 2KB PSUM bank; host sums.
            acc = accpool.tile([WIN, 4, 512], f32)

            # fill: stage1 tile t feeds stage2-chunk0 tile t immediately, so
            # PE work and evacuations pipeline through the fill instead of
            # serializing. Then per chunk: segment MMs with the next chunk's
            # stage2+evac interleaved every 32 rows so a PSUM-bank wait
            # never head-of-line-blocks the segment stream.
            ALL0 = allpool.tile([WIN, NV, RHALF], bf16, tag="all", name="ALL0")
            ALLt = {0: ALL0}
            for t in range(8):
                emit_stage1_tile(t)
            for t in range(8):
                emit_stage2_tile(ALLt[0], 0, t)
            for m in range(MCH):
                if m + 1 < MCH:
                    ALLt[m + 1] = allpool.tile(
                        [WIN, NV, RHALF], bf16, tag="all", name=f"ALL{m+1}"
                    )
                ALL = ALLt[m]
                for r in range(RHALF):
                    g = r % 4
                    nc.tensor.matmul(
                        acc[32 * g : 32 * g + K, g, 0:NV],
                        OH_sb[:, m, r * K : r * K + K].bitcast(f8),
                        ALL[:, :, r],
                        start=(m == 0 and r < 4),
                        stop=(m == MCH - 1 and r >= RHALF - 4),
                        tile_position=(0, 32 * g),
                        skip_group_check=True,
                    )
                    if m + 1 < MCH and r >= 12 and (r - 12) % 26 == 0 and (
                        r - 12
                    ) // 26 < 8:
                        emit_stage2_tile(ALLt[m + 1], m + 1, (r - 12) // 26)

            out_sb = outpool.tile([WIN, 4, NV], f32)
            nc.vector.memset(out_sb[:], 0.0)
            for g in range(4):
                nc.vector.tensor_copy(
                    out_sb[32 * g : 32 * g + K, g, :],
                    acc[32 * g : 32 * g + K, g, 0:NV],
                )
            nc.sync.dma_start(out=out[:], in_=out_sb[:])


_CACHED = None


def _patch_ldw_opt():
    """Enable walrus's LDWEIGHTS optimization pass (dedupes redundant
    stationary-operand reloads); concourse hardcodes it off."""
    import concourse.bass_utils as bu

    if getattr(bu, "_ldw_opt_patched", False):
        return
    orig = bu.run_command

    def patched(cmd, **kw):
        if isinstance(cmd, list):
            cmd = [
                "--enable-ldw-opt=true" if c == "--enable-ldw-opt=false" else c
                for c in cmd
            ]
        return orig(cmd, **kw)

    bu.run_command = patched
    bu._ldw_opt_patched = True


def _strip_segment_ldws(nc, mybir):
    """Remove the per-matmul 19-col fp8 LDWEIGHTS that tile emits for the
    segment matmuls — the quad LDWEIGHTS already loaded those columns.
    Their semaphore updates/waits are merged into the next instruction so
    every downstream threshold still adds up."""
    f = list(nc.m.functions)[0]
    for blk in f.blocks:
        il = list(blk.instructions)
        kept = []
        pend_waits, pend_upds = [], []
        removed = 0
        for ins in il:
            if type(ins).__name__ == "InstLdweights":
                ap = list(ins.ins)[0]
                if str(ap.dtype) == "dt.float8e4" and ap.ap[-1][1] == K:
                    si = ins.sync_info
                    if si is not None:
                        pend_waits.extend(si.on_wait)
                        pend_upds.extend(si.on_update)
                    removed += 1
                    continue
            if pend_waits or pend_upds:
                si = ins.sync_info
                w = list(si.on_wait) if si else []
                u = list(si.on_update) if si else []
                ins.sync_info = mybir.SyncInfo(
                    on_wait=pend_waits + w, on_update=pend_upds + u
                )
                pend_waits, pend_upds = [], []
            kept.append(ins)
        assert not pend_waits and not pend_upds
        if removed:
            blk.instructions = kept


def _build_nc():
    global _CACHED
    if _CACHED is not None:
        return _CACHED
    import concourse.bacc as bacc
    import concourse.tile as tile
    import concourse.mybir as mybir

    f32 = mybir.dt.float32
    bf16 = mybir.dt.bfloat16
    f8 = mybir.dt.float8e4
    nc = bacc.Bacc("TRN2", target_bir_lowering=False, debug=False)
    x = nc.dram_tensor("x", (HS, C, WIN), bf16, kind="ExternalInput")
    aux = nc.dram_tensor("aux", (WIN, AUXW), bf16, kind="ExternalInput")
    oh = nc.dram_tensor(
        "oh", (MCH, WIN, RHALF * K), mybir.dt.uint8, kind="ExternalInput"
    )
    out = nc.dram_tensor("out", (WIN, 4, NV), f32, kind="ExternalOutput")
    _trace_device_kernel(nc, tile, mybir, x, aux, oh, out)
    nc.compile()
    _CACHED = nc
    return nc


def make_in_maps(embedding, label):
    """Shard the full inputs into the 8 per-core input dicts."""
    import ml_dtypes

    U = _resize_weight_mat(HIN, HOUT)  # (512, 128)
    kvec = np.arange(K, dtype=np.int64)
    in_maps = []
    for n in range(N_IMAGES):
        for half in range(2):
            r0, h0 = (0, 0) if half == 0 else (RHALF, HIN - HS)
            lab = np.asarray(label[n, r0 : r0 + RHALF, :])
            aux = np.zeros((WIN, AUXW), np.float32)
            aux[0:HS, 0:RHALF] = U[r0 : r0 + RHALF, h0 : h0 + HS].T
            aux[:, RHALF:AUXW] = U.T
            # one-hot, exact in fp8e4m3 (byte 0x38 == 1.0)
            ohm = np.zeros((MCH, WIN, RHALF, K), np.uint8)
            for m in range(MCH):
                lm = lab[:, m * WIN : (m + 1) * WIN].T  # (128, 256)
                ohm[m] = (lm[:, :, None] == kvec) * np.uint8(0x38)
            in_maps.append(
                {
                    "x": np.ascontiguousarray(
                        embedding[n, :, h0 : h0 + HS, :].transpose(1, 0, 2)
                    ).astype(ml_dtypes.bfloat16),
                    "aux": aux.astype(ml_dtypes.bfloat16),
                    "oh": ohm.reshape(MCH, WIN, RHALF * K),
                }
            )
    return in_maps


def combine(partials, label):
    """Host epilogue: 8 x (128, 4, 64) partials + exact label histogram
    -> (4,) loss, replicating the reference formulas from the per-class
    sufficient statistics."""
    label = np.asarray(label)
    out = np.zeros(N_IMAGES, np.float32)
    for n in range(N_IMAGES):
        tot = np.zeros((K, NV), np.float64)
        for p in (partials[2 * n], partials[2 * n + 1]):
            p = p.astype(np.float64)
            for g in range(4):
                tot += p[32 * g : 32 * g + K, g, :]
        S1 = tot[:, :C]                 # (K, C) per-class embedding sums
        S2 = tot[:, C : 2 * C].sum(1)   # (K,) per-class sum of squared norms
        count = np.bincount(label[n].reshape(-1), minlength=K).astype(
            np.float64
        )
        mask = (count > 0).astype(np.float64)
        mean = S1 / (count[:, None] + 1.0)
        intra = (
            (S2 - 2 * (mean * S1).sum(1) + count * (mean * mean).sum(1))
            / C
            / (count + 1.0)
        )
        n_fg = mask[1:].sum()
        l2_intra = (intra[1:] * mask[1:]).sum() / n_fg
        diff = mean[:, None, :] - mean[None, :, :]
        inter = (diff**2).mean(-1) * mask[None, :] * mask[:, None]
        l2_inter = inter[1:, 1:].sum() / (n_fg * n_fg)
        out[n] = l2_intra - l2_inter
    return out


def kernel(embedding, label):
    from concourse.bass_utils import run_bass_kernel_spmd

    nc = _build_nc()
    embedding = np.asarray(embedding)
    label = np.asarray(label)
    in_maps = make_in_maps(embedding, label)
    res = run_bass_kernel_spmd(nc, in_maps, list(range(NCORES)))
    partials = [res.results[i]["out"] for i in range(NCORES)]
    return combine(partials, label)


# revision 44
# speedup vs baseline: 1.1781x; 1.1781x over previous
"""Trainium2 Bass kernel for nn_DiscriminativeLoss (segment_reduce).

Strategy (pure data parallel, 8 cores = 4 images x 2 half-images):
  Each core handles 256 output rows (half of a 512x512 image) for one image.
  On device (per core):
    - stage 1 (PE): row-upsample  B[w, c, R] = sum_h X[h,c,w] * Ur[h,R]
    - stage 2 (PE): col-upsample  EU[CC, c, R] = sum_w Uc[w,CC] * B[w,c,R]
    - evac (ACT+DVE split, batched 2 PSUM banks/op): EU copy -> ALL[:, r, 0:32]
      and EU^2 (DVE mult / ACT square) -> ALL[:, r, 32:64], r-major bf16.
    - PE segment-reduce: for each 128-pixel tile (row r, col chunk m):
        acc[k, :] += onehot[pix, k]^T @ ALL[pix, r, :]   (N=64)
      one-hot ships from HOST as exact fp8 0/1 (pure function of the label
      input); 4 independent accumulators in PE col-groups (tile_position).
  Host: per-class counts via bincount (exact), then combines the 8 (19,64)
  partials into per-class count / sum / sum-of-squares and evaluates the
  tiny closed-form loss exactly as the reference.

The bilinear-resize weight matrix replicates jax.image.resize (triangle
kernel, half-pixel centers, edge renormalization) and is fed to the device,
so the upsample is the exact same linear operator as the reference.
"""

import numpy as np

N_IMAGES = 4
C = 32
HIN = WIN = 128
HOUT = WOUT = 512
K = 19          # n_classes
RHALF = 256     # output rows per core
HS = 65         # input rows per core (with halo)
NV = 2 * C      # rhs value columns: 32 emb + 32 emb^2
MCH = 4         # output column chunks of 128
NCORES = 8
AUXW = RHALF + WOUT  # 768: WR cols then WC cols


def _resize_weight_mat(in_size, out_size):
    """(out, in) weight matrix of jax.image.resize(..., method='bilinear')."""
    scale = out_size / in_size
    inv_scale = 1.0 / scale
    sample_f = (np.arange(out_size, dtype=np.float32) + 0.5) * inv_scale - 0.5
    x = np.abs(sample_f[None, :] - np.arange(in_size, dtype=np.float32)[:, None])
    weights = np.maximum(0, 1 - x)
    total = weights.sum(axis=0, keepdims=True)
    weights = np.where(
        np.abs(total) > 1000.0 * np.finfo(np.float32).eps,
        weights / np.where(total != 0, total, 1),
        0,
    )
    keep = (sample_f >= -0.5) & (sample_f <= in_size - 0.5)
    weights = np.where(keep[None, :], weights, 0)
    return np.ascontiguousarray(weights.T.astype(np.float32))  # (out, in)


def _trace_device_kernel(nc, tile, mybir, x, aux, oh, out):
    from contextlib import ExitStack

    f32 = mybir.dt.float32
    bf16 = mybir.dt.bfloat16
    f8 = mybir.dt.float8e4
    with tile.TileContext(nc) as tc:
        with ExitStack() as ctx:
            consts = ctx.enter_context(tc.tile_pool(name="consts", bufs=1))
            AUX = consts.tile([WIN, AUXW], bf16)
            nc.sync.dma_start(out=AUX[:, 0:RHALF], in_=aux[:, 0:RHALF])
            X_sb = consts.tile([HS, C, WIN], bf16)
            nc.sync.dma_start(out=X_sb[:], in_=x[:])
            nc.scalar.dma_start(out=AUX[:, RHALF:AUXW], in_=aux[:, RHALF:AUXW])
            WR_sb = AUX[0:HS, 0:RHALF]
            # one-hot ships as uint8 bytes (jax bridge lacks fp8e4m3);
            # bitcast to fp8 at the matmul weight read.
            OH_sb = consts.tile([WIN, MCH, RHALF * K], mybir.dt.uint8)
            for m in range(MCH):
                nc.sync.dma_start(out=OH_sb[:, m, :], in_=oh[m, :, :])

            B = consts.tile([WIN, C, RHALF], bf16)
            # 2-bank psum tiles: two matmuls (one per bank half) per tile,
            # then one batched 2-bank evacuation per engine.
            ps2 = ctx.enter_context(
                tc.tile_pool(name="ps2", bufs=2, space="PSUM")
            )
            # warm the PE HAM clock gate during the input-DMA wait:
            # dependency-free matmuls on a memset tile bring the PE to
            # 2.4 GHz before stage 1 issues.
            WARM = consts.tile([WIN, WIN], bf16)
            nc.vector.memset(WARM[:], 0.0)
            for w in range(16):
                pw = ps2.tile([WIN, 4, RHALF], f32, tag="ps2")
                nc.tensor.matmul(
                    pw[0:1, 0, 0:64], WARM[:, 0:1], WARM[:, 0:64],
                    start=True, stop=True,
                )

            def emit_stage1_tile(t):
                """Row-upsample channels 4t..4t+4 into B."""
                c = 4 * t
                p = ps2.tile([WIN, 4, RHALF], f32, tag="ps2")
                for j in range(4):
                    nc.tensor.matmul(
                        p[:, j, :], X_sb[:, c + j, :], WR_sb[:],
                        start=True, stop=True,
                    )
                if t % 2 == 0:
                    nc.vector.tensor_copy(B[:, c : c + 4, :], p[:])
                else:
                    nc.scalar.copy(B[:, c : c + 4, :], p[:])

            def emit_stage2_tile(ALLm, m, t):
                """Col-upsample channels 4t..4t+4 of chunk m into ALLm,
                evacuate (copy) + square. DVE copies 2/8, ACT 6/8; all
                squares on DVE (2x on contiguous bf16)."""
                cg = 4 * t
                WC_m = AUX[:, RHALF + m * WIN : RHALF + (m + 1) * WIN]
                p = ps2.tile([WIN, 4, RHALF], f32, tag="ps2")
                for j in range(0, 4, 2):
                    nc.tensor.matmul(
                        p[:, j : j + 2, :],
                        WC_m,
                        B[:, cg + j : cg + j + 2, :],
                        start=True, stop=True,
                    )
                euv = ALLm[:, cg : cg + 4, :]
                sqv = ALLm[:, C + cg : C + cg + 4, :]
                if t == 0:
                    nc.vector.tensor_copy(euv, p[:])
                else:
                    nc.scalar.copy(euv, p[:])
                nc.vector.tensor_tensor(
                    out=sqv, in0=euv, in1=euv, op=mybir.AluOpType.mult
                )

            allpool = ctx.enter_context(tc.tile_pool(name="allpool", bufs=2))
            accpool = ctx.enter_context(
                tc.tile_pool(name="accpool", bufs=1, space="PSUM")
            )
            outpool = ctx.enter_context(tc.tile_pool(name="outpool", bufs=1))
            # 4 independent accumulators in col-groups 0..3 of the PE array
            # (tile_position packing): group g = r % 4 accumulates into
            # partitions [32g, 32g+19) of its own 2KB PSUM bank; host sums.
            acc = accpool.tile([WIN, 4, 512], f32)

            # fill: stage1 tile t feeds stage2-chunk0 tile t immediately, so
            # PE work and evacuations pipeline through the fill instead of
            # serializing. Then per chunk: segment MMs with the next chunk's
            # stage2+evac interleaved every 32 rows so a PSUM-bank wait
            # never head-of-line-blocks the segment stream.
            ALL0 = allpool.tile([WIN, NV, RHALF], bf16, tag="all", name="ALL0")
            ALLt = {0: ALL0}
            # fill: stage2-chunk0 trails stage1 by 4 tiles, so its inputs
            # (stage1 evacs) are already landed and both stages pipeline.
            for t in range(8):
                emit_stage1_tile(t)
                if t >= 4:
                    emit_stage2_tile(ALLt[0], 0, t - 4)
            for t in range(4, 8):
                emit_stage2_tile(ALLt[0], 0, t)
            for m in range(MCH):
                if m + 1 < MCH:
                    ALLt[m + 1] = allpool.tile(
                        [WIN, NV, RHALF], bf16, tag="all", name=f"ALL{m+1}"
                    )
                ALL = ALLt[m]
                for r in range(RHALF):
                    g = r % 4
                    nc.tensor.matmul(
                        acc[32 * g : 32 * g + K, g, 0:NV],
                        OH_sb[:, m, r * K : r * K + K].bitcast(f8),
                        ALL[:, :, r],
                        start=(m == 0 and r < 4),
                        stop=(m == MCH - 1 and r >= RHALF - 4),
                        tile_position=(0, 32 * g),
                        skip_group_check=True,
                    )
                    if m + 1 < MCH and r >= 12 and (r - 12) % 26 == 0 and (
                        r - 12
                    ) // 26 < 8:
                        emit_stage2_tile(ALLt[m + 1], m + 1, (r - 12) // 26)

            out_sb = outpool.tile([WIN, 4, NV], f32)
            nc.vector.memset(out_sb[:], 0.0)
            for g in range(4):
                nc.vector.tensor_copy(
                    out_sb[32 * g : 32 * g + K, g, :],
                    acc[32 * g : 32 * g + K, g, 0:NV],
                )
            nc.sync.dma_start(out=out[:], in_=out_sb[:])


_CACHED = None


def _patch_ldw_opt():
    """Enable walrus's LDWEIGHTS optimization pass (dedupes redundant
    stationary-operand reloads); concourse hardcodes it off."""
    import concourse.bass_utils as bu

    if getattr(bu, "_ldw_opt_patched", False):
        return
    orig = bu.run_command

    def patched(cmd, **kw):
        if isinstance(cmd, list):
            cmd = [
                "--enable-ldw-opt=true" if c == "--enable-ldw-opt=false" else c
                for c in cmd
            ]
        return orig(cmd, **kw)

    bu.run_command = patched
    bu._ldw_opt_patched = True


def _strip_segment_ldws(nc, mybir):
    """Remove the per-matmul 19-col fp8 LDWEIGHTS that tile emits for the
    segment matmuls — the quad LDWEIGHTS already loaded those columns.
    Their semaphore updates/waits are merged into the next instruction so
    every downstream threshold still adds up."""
    f = list(nc.m.functions)[0]
    for blk in f.blocks:
        il = list(blk.instructions)
        kept = []
        pend_waits, pend_upds = [], []
        removed = 0
        for ins in il:
            if type(ins).__name__ == "InstLdweights":
                ap = list(ins.ins)[0]
                if str(ap.dtype) == "dt.float8e4" and ap.ap[-1][1] == K:
                    si = ins.sync_info
                    if si is not None:
                        pend_waits.extend(si.on_wait)
                        pend_upds.extend(si.on_update)
                    removed += 1
                    continue
            if pend_waits or pend_upds:
                si = ins.sync_info
                w = list(si.on_wait) if si else []
                u = list(si.on_update) if si else []
                ins.sync_info = mybir.SyncInfo(
                    on_wait=pend_waits + w, on_update=pend_upds + u
                )
                pend_waits, pend_upds = [], []
            kept.append(ins)
        assert not pend_waits and not pend_upds
        if removed:
            blk.instructions = kept


def _build_nc():
    global _CACHED
    if _CACHED is not None:
        return _CACHED
    import concourse.bacc as bacc
    import concourse.tile as tile
    import concourse.mybir as mybir

    f32 = mybir.dt.float32
    bf16 = mybir.dt.bfloat16
    f8 = mybir.dt.float8e4
    nc = bacc.Bacc("TRN2", target_bir_lowering=False, debug=False)
    x = nc.dram_tensor("x", (HS, C, WIN), bf16, kind="ExternalInput")
    aux = nc.dram_tensor("aux", (WIN, AUXW), bf16, kind="ExternalInput")
    oh = nc.dram_tensor(
        "oh", (MCH, WIN, RHALF * K), mybir.dt.uint8, kind="ExternalInput"
    )
    out = nc.dram_tensor("out", (WIN, 4, NV), f32, kind="ExternalOutput")
    _trace_device_kernel(nc, tile, mybir, x, aux, oh, out)
    nc.compile()
    _CACHED = nc
    return nc


def make_in_maps(embedding, label):
    """Shard the full inputs into the 8 per-core input dicts."""
    import ml_dtypes

    U = _resize_weight_mat(HIN, HOUT)  # (512, 128)
    kvec = np.arange(K, dtype=np.int64)
    in_maps = []
    for n in range(N_IMAGES):
        for half in range(2):
            r0, h0 = (0, 0) if half == 0 else (RHALF, HIN - HS)
            lab = np.asarray(label[n, r0 : r0 + RHALF, :])
            aux = np.zeros((WIN, AUXW), np.float32)
            aux[0:HS, 0:RHALF] = U[r0 : r0 + RHALF, h0 : h0 + HS].T
            aux[:, RHALF:AUXW] = U.T
            # one-hot, exact in fp8e4m3 (byte 0x38 == 1.0)
            ohm = np.zeros((MCH, WIN, RHALF, K), np.uint8)
            for m in range(MCH):
                lm = lab[:, m * WIN : (m + 1) * WIN].T  # (128, 256)
                ohm[m] = (lm[:, :, None] == kvec) * np.uint8(0x38)
            in_maps.append(
                {
                    "x": np.ascontiguousarray(
                        embedding[n, :, h0 : h0 + HS, :].transpose(1, 0, 2)
                    ).astype(ml_dtypes.bfloat16),
                    "aux": aux.astype(ml_dtypes.bfloat16),
                    "oh": ohm.reshape(MCH, WIN, RHALF * K),
                }
            )
    return in_maps


def combine(partials, label):
    """Host epilogue: 8 x (128, 4, 64) partials + exact label histogram
    -> (4,) loss, replicating the reference formulas from the per-class
    sufficient statistics."""
    label = np.asarray(label)
    out = np.zeros(N_IMAGES, np.float32)
    for n in range(N_IMAGES):
        tot = np.zeros((K, NV), np.float64)
        for p in (partials[2 * n], partials[2 * n + 1]):
            p = p.astype(np.float64)
            for g in range(4):
                tot += p[32 * g : 32 * g + K, g, :]
        S1 = tot[:, :C]                 # (K, C) per-class embedding sums
        S2 = tot[:, C : 2 * C].sum(1)   # (K,) per-class sum of squared norms
        count = np.bincount(label[n].reshape(-1), minlength=K).astype(
            np.float64
        )
        mask = (count > 0).astype(np.float64)
        mean = S1 / (count[:, None] + 1.0)
        intra = (
            (S2 - 2 * (mean * S1).sum(1) + count * (mean * mean).sum(1))
            / C
            / (count + 1.0)
        )
        n_fg = mask[1:].sum()
        l2_intra = (intra[1:] * mask[1:]).sum() / n_fg
        diff = mean[:, None, :] - mean[None, :, :]
        inter = (diff**2).mean(-1) * mask[None, :] * mask[:, None]
        l2_inter = inter[1:, 1:].sum() / (n_fg * n_fg)
        out[n] = l2_intra - l2_inter
    return out


def kernel(embedding, label):
    from concourse.bass_utils import run_bass_kernel_spmd

    nc = _build_nc()
    embedding = np.asarray(embedding)
    label = np.asarray(label)
    in_maps = make_in_maps(embedding, label)
    res = run_bass_kernel_spmd(nc, in_maps, list(range(NCORES)))
    partials = [res.results[i]["out"] for i in range(NCORES)]
    return combine(partials, label)
